# revision 18
# baseline (speedup 1.0000x reference)
"""Multi-head causal attention (B=8, S=1024, E=512, H=8, Dk=Dv=64) on 8 NeuronCores.

Sharding: data-parallel over batch. Core b computes the full attention block
for X[b]; no collectives. Host pre-transposes X[b] -> [E, S], converts matmul
operands to bf16, and packs weights into wide slabs so the device issues only
7 input DMAs (scalar + sync HW queues in compute order).

Per-core dataflow (bf16 matmuls, fp32 PSUM accumulate / softmax math):
  XT slab [128, qc*2048 + c*512 + s] resident in SBUF
  QT/KT per head-pair per q-half: [128 dd, 512 q] (W chunk stationary)
  V = (X @ Wv + bv) -> 8 tiles [128 s, 8*(64+1)] with a ones column per head
      so the AV matmul also emits softmax denominators
  attention per (pair, q-chunk), software-pipelined one k-block ahead:
    score^T blocks [128 k, 2x512 q] on PE (k-blocks above the diagonal
    skipped, partial blocks column-trimmed), causal staircase mask added on
    the Pool engine directly in PSUM (no PE fixup matmuls), exp on ScalarE
    (scale=1/8 folded), AV accum O^T[65, q] with denom row 64.
    Emission order score(ki+1) before AV(ki) keeps PE busy during exp.
  finalize: reciprocal(denom) on DVE straight from PSUM, Pool partition-
    broadcast, DVE scale; head-pair upper half placed via SBUF->SBUF DMA.
  Y[s-chunk] = sum_p O_pair^T-block^T @ Wo + bo, emitted as soon as the
  needed q-chunk's pairs are finalized (first half overlaps qc=1 attention);
  y stores stream out on the sync queue throughout.
"""

import numpy as np
import ml_dtypes

import concourse.bass as bass
import concourse.tile as tile
import concourse.mybir as mybir
from concourse import bacc
from concourse import bass_utils

B, S, E = 8, 1024, 512
H, DK, DV = 8, 64, 64
HD = H * DK  # 512
P = 128
EC = E // P  # 4 contraction chunks over E
NPAIR = H // 2
NCORES = 8
F32 = mybir.dt.float32
BF16 = mybir.dt.bfloat16
NEG = -1.0e9

_COMPILED = None


def _body(nc, tc, const, work, ps, pb, d):
    # ---- const tiles + packed input DMAs ----
    XT = const.tile([P, 4096], BF16, tag="xt", name="XT")
    WQ = const.tile([P, 2048], BF16, tag="wq", name="WQ")
    WK = const.tile([P, 2048], BF16, tag="wk", name="WK")
    WV = const.tile([P, 2048], BF16, tag="wv", name="WV")
    WO = const.tile([P, 2048], BF16, tag="wo", name="WO")
    CB = const.tile([P, 1408], BF16, tag="cb", name="CB")
    FB = const.tile([P, 8], F32, tag="fb", name="FB")

    # critical prefix (QT/KT qc=0 deps) on the sync HW queue; the rest
    # streams in parallel on the scalar engine's HW queue
    nc.sync.dma_start(WQ[:], d["wq"][:])
    nc.sync.dma_start(XT[:, 0:2048], d["xt"][:, 0:2048])
    nc.sync.dma_start(WK[:], d["wk"][:])
    nc.sync.dma_start(XT[:, 2048:4096], d["xt"][:, 2048:4096])
    nc.scalar.dma_start(FB[:], d["fb"][:])
    nc.scalar.dma_start(CB[:], d["cb"][:])
    nc.scalar.dma_start(WV[:], d["wv"][:])
    nc.scalar.dma_start(WO[:], d["wo"][:])

    bq_t = FB[:, 0:4]
    bk_t = FB[:, 4:8]
    negi = CB[:, 0:128]
    tri2 = CB[:, 128:384]
    bvb = CB[:, 384:896]
    bob = CB[:, 896:1408]

    qt, kt, ot_sb = {}, {}, {}
    v_sb = [None] * 8

    def emit_qkt(p, qc):
        for W, bias, store, nm in ((WQ, bq_t, qt, "q"), (WK, bk_t, kt, "k")):
            pp = ps.tile([P, 512], F32, tag="ps", name=f"{nm}p{p}{qc}")
            for c in range(EC):
                nc.tensor.matmul(
                    pp[:], W[:, c * 512 + p * P:c * 512 + (p + 1) * P],
                    XT[:, qc * 2048 + c * 512:qc * 2048 + (c + 1) * 512],
                    start=(c == 0), stop=(c == EC - 1))
            t = const.tile([P, 512], BF16, tag=f"{nm}t{p}{qc}", name=f"{nm}t{p}{qc}")
            nc.scalar.activation(
                t[:], pp[:], mybir.ActivationFunctionType.Identity,
                bias=bias[:, p:p + 1])
            store[p, qc] = t

    def emit_v(si):
        qc, sl = divmod(si, 4)
        vp = ps.tile([P, HD], F32, tag="ps", name=f"vp{si}")
        for c in range(EC):
            base = qc * 2048 + c * 512
            nc.tensor.matmul(
                vp[:], XT[:, base + sl * P:base + (sl + 1) * P],
                WV[:, c * 512:(c + 1) * 512],
                start=(c == 0), stop=(c == EC - 1))
        t = const.tile([P, H * 65], BF16, tag=f"v{si}", name=f"v{si}")
        t3 = t.rearrange("p (h c) -> p h c", c=65)
        nc.gpsimd.memset(t[:], 1.0)  # contiguous; leaves the per-head ones column
        nc.vector.tensor_add(
            t3[:, :, 0:DV],
            vp.rearrange("p (h c) -> p h c", c=DV),
            bvb.rearrange("p (h c) -> p h c", c=DV))
        v_sb[si] = t

    def emit_proj(si):
        qc, sl = divmod(si, 4)
        yp = ps.tile([P, E], F32, tag="ps", name=f"yp{si}")
        for p in range(NPAIR):
            nc.tensor.matmul(
                yp[:], ot_sb[p, qc][:, sl * P:(sl + 1) * P],
                WO[:, p * 512:(p + 1) * 512],
                start=(p == 0), stop=(p == NPAIR - 1))
        yo = work.tile([P, E], F32, tag="yo", name=f"yo{si}", bufs=2)
        nc.vector.tensor_add(yo[:], yp[:], bob[:])
        nc.sync.dma_start(d["y"][si * P:(si + 1) * P, :], yo[:])

    def attn(p, qc):
        n_ki = 4 * (qc + 1)
        otp = [ps.tile([DV + 1, 512], F32, tag="ps", name=f"otp{p}{qc}{hb}")
               for hb in (0, 1)]
        stps, stes = {}, {}

        def emit_score(ki):
            kc, kl = divmod(ki, 4)
            diag = (ki * P - qc * 512) >= 0
            off = max(ki * P - qc * 512, 0)
            stp = pb.tile([P, 1024], F32, tag="st", name=f"st{p}{qc}{ki}")
            for hb in (0, 1):
                hp = slice(hb * DK, (hb + 1) * DK)
                nc.tensor.matmul(
                    stp[:, hb * 512 + off:(hb + 1) * 512],
                    kt[p, kc][hp, kl * P:(kl + 1) * P],
                    qt[p, qc][hp, off:],
                    start=True, stop=not diag, tile_position=(hb * DK, 0),
                    skip_group_check=True)
            if diag:
                # causal triangle applied on the PE itself (-1e9*I @ tri01
                # accumulated into the diagonal 128 columns)
                for hb in (0, 1):
                    nc.tensor.matmul(
                        stp[:, hb * 512 + off:hb * 512 + off + P],
                        negi[:], tri2[:, 0:P],
                        start=False, stop=True, skip_group_check=True)
            stps[ki] = (stp, off)

        def emit_exp(ki):
            stp, off = stps[ki]
            ste = work.tile([P, 1024], BF16, tag="ste", name=f"ste{p}{qc}{ki}",
                            bufs=3)
            stp3 = stp.rearrange("p (h q) -> p h q", h=2)[:, :, off:]
            ste3 = ste.rearrange("p (h q) -> p h q", h=2)[:, :, off:]
            nc.scalar.activation(
                ste3, stp3, mybir.ActivationFunctionType.Exp, scale=0.125)
            stes[ki] = (ste, off)

        def emit_av(ki):
            ste, off = stes[ki]
            st_f, sp_f = (ki == 0), (ki == n_ki - 1)
            for hb in (0, 1):
                h = 2 * p + hb
                nc.tensor.matmul(
                    otp[hb][:, off:], v_sb[ki][:, h * 65:h * 65 + 65],
                    ste[:, hb * 512 + off:(hb + 1) * 512],
                    start=st_f, stop=sp_f, skip_group_check=True)

        emit_score(0)
        emit_exp(0)
        for ki in range(1, n_ki):
            emit_score(ki)
            emit_exp(ki)
            emit_av(ki - 1)
        emit_av(n_ki - 1)

        # ---- finalize: O^T *= 1/denom (denom in row DV of otp); per-head
        # chains interleaved so DVE/GpSimd/DMA stages pipeline ----
        ot = const.tile([P, 512], BF16, tag=f"ot{p}{qc}", name=f"ot{p}{qc}")
        rec, rb = {}, {}
        for hb in (0, 1):
            rrow = work.tile([1, 512], F32, tag="rrow", name=f"rrow{p}{qc}{hb}",
                             bufs=2)
            nc.vector.tensor_copy(rrow[:], otp[hb][DV:DV + 1, :])
            rec[hb] = work.tile([1, 512], F32, tag="rec", name=f"rec{p}{qc}{hb}",
                                bufs=2)
            nc.vector.reciprocal_approx_fast(rec[hb][:], rrow[:])
            rb[hb] = work.tile([DV, 512], F32, tag="rb", name=f"rb{p}{qc}{hb}",
                               bufs=2)
            nc.gpsimd.partition_broadcast(rb[hb][:], rec[hb][:])
        nc.vector.tensor_mul(ot[0:DV, :], otp[0][0:DV, :], rb[0][:])
        # DVE cannot shift partitions: scale into a temp at base 0, then
        # SBUF->SBUF DMA into partitions 64-127 of the pair tile
        tmp = work.tile([DV, 512], BF16, tag="ottmp",
                        name=f"ottmp{p}{qc}", bufs=2)
        nc.vector.tensor_mul(tmp[:], otp[1][0:DV, :], rb[1][:])
        nc.sync.dma_start(ot[DV:P, :], tmp[:])
        ot_sb[p, qc] = ot

    # ---- emission schedule (ps-tag rotation keeps otp pairs ping-ponging
    # between bank pairs; see allocation count comments) ----
    for p in range(NPAIR):            # ps allocs 0-7
        emit_qkt(p, 0)
    for si in range(4):               # 8-11
        emit_v(si)
    for p in range(NPAIR):            # 12-19 -> slots (0,1)/(2,3)/(0,1)/(2,3)
        attn(p, 0)
    emit_qkt(0, 1)                    # 20-21
    emit_qkt(1, 1)                    # 22-23
    emit_v(4)                         # 24
    emit_v(5)                         # 25
    emit_qkt(2, 1)                    # 26-27
    emit_qkt(3, 1)                    # 28-29
    emit_v(6)                         # 30
    emit_v(7)                         # 31
    for si in range(4):               # 32-35 (needs all qc=0 ot, finalized)
        emit_proj(si)
    for p in range(NPAIR):            # 36-43 -> slots (0,1)/(2,3)/(0,1)/(2,3)
        attn(p, 1)
    for si in range(4, 8):            # 44-47
        emit_proj(si)


def _build():
    nc = bacc.Bacc("TRN2", target_bir_lowering=False, debug=False)
    d = {
        "xt": nc.dram_tensor("xt", [P, 4096], BF16, kind="ExternalInput").ap(),
        "wq": nc.dram_tensor("wq", [P, 2048], BF16, kind="ExternalInput").ap(),
        "wk": nc.dram_tensor("wk", [P, 2048], BF16, kind="ExternalInput").ap(),
        "wv": nc.dram_tensor("wv", [P, 2048], BF16, kind="ExternalInput").ap(),
        "wo": nc.dram_tensor("wo", [P, 2048], BF16, kind="ExternalInput").ap(),
        "cb": nc.dram_tensor("cb", [P, 1408], BF16, kind="ExternalInput").ap(),
        "fb": nc.dram_tensor("fb", [P, 8], F32, kind="ExternalInput").ap(),
        "y": nc.dram_tensor("y", [S, E], F32, kind="ExternalOutput").ap(),
    }
    with tile.TileContext(nc) as tc:
        with tc.tile_pool(name="const", bufs=1) as const, \
             tc.tile_pool(name="work", bufs=3) as work, \
             tc.tile_pool(name="ps", bufs=4, space="PSUM") as ps, \
             tc.tile_pool(name="pb", bufs=2, space="PSUM") as pb:
            _body(nc, tc, const, work, ps, pb, d)
    nc.compile()
    return nc


def get_nc():
    global _COMPILED
    if _COMPILED is None:
        _COMPILED = _build()
    return _COMPILED


def _prep_in_maps(X, Wq, bq, Wk, bk, Wv, bv, Wo, bo):
    f = np.float32
    bf = ml_dtypes.bfloat16

    def wslab(W):  # [H,E,Dk] -> [128, c*512 + (h*64+d)]
        Wr = np.transpose(np.asarray(W, f), (1, 0, 2)).reshape(E, HD)
        return np.ascontiguousarray(
            Wr.reshape(EC, P, HD).transpose(1, 0, 2).reshape(P, EC * HD).astype(bf))

    shared = {
        "wq": wslab(Wq),
        "wk": wslab(Wk),
        "wv": wslab(Wv),
        "wo": np.ascontiguousarray(
            np.asarray(Wo, f).reshape(EC, P, E).transpose(1, 0, 2)
            .reshape(P, EC * E).astype(bf)),
    }
    bq_t = np.asarray(bq, f).reshape(HD).reshape(NPAIR, P).T
    bk_t = np.asarray(bk, f).reshape(HD).reshape(NPAIR, P).T
    bvb = np.broadcast_to(np.asarray(bv, f).reshape(1, HD), (P, HD)).astype(bf)
    bob = np.broadcast_to(np.asarray(bo, f).reshape(1, E), (P, E)).astype(bf)
    kk = np.arange(P)[:, None]
    jj = np.arange(P)[None, :]
    shared["fb"] = np.ascontiguousarray(
        np.concatenate([bq_t, bk_t], axis=1).astype(f))
    negi = (np.eye(P, dtype=f) * NEG).astype(bf)
    tri01 = (kk > jj).astype(bf)
    shared["cb"] = np.ascontiguousarray(
        np.concatenate([negi, tri01, tri01, bvb, bob], axis=1))

    Xf = np.asarray(X, f)
    in_maps = []
    for b in range(B):
        m = dict(shared)
        # xt slab: [128, qc*2048 + c*512 + s']
        m["xt"] = np.ascontiguousarray(
            Xf[b].T.reshape(EC, P, 2, 512).transpose(1, 2, 0, 3)
            .reshape(P, 4096).astype(bf))
        in_maps.append(m)
    return in_maps


def kernel(X, Wq, bq, Wk, bk, Wv, bv, Wo, bo):
    nc = get_nc()
    in_maps = _prep_in_maps(X, Wq, bq, Wk, bk, Wv, bv, Wo, bo)
    res = bass_utils.run_bass_kernel_spmd(nc, in_maps, core_ids=list(range(NCORES)))
    return np.stack([res.results[b]["y"] for b in range(B)], axis=0).astype(np.float32)


def run_traced(X, Wq, bq, Wk, bk, Wv, bv, Wo, bo):
    """Like kernel() but with NTFF profiling; returns (out, exec_time_ns)."""
    nc = get_nc()
    in_maps = _prep_in_maps(X, Wq, bq, Wk, bk, Wv, bv, Wo, bo)
    res = bass_utils.run_bass_kernel_spmd(
        nc, in_maps, core_ids=list(range(NCORES)), trace=True)
    out = np.stack([res.results[b]["y"] for b in range(B)], axis=0).astype(np.float32)
    return out, res.exec_time_ns


# revision 68
# speedup vs baseline: 1.1907x; 1.1907x over previous
"""Multi-head causal attention (B=8, S=1024, E=512, H=8, Dk=Dv=64) on 8 NeuronCores.

Sharding: data-parallel over batch. Core b computes the full attention block
for X[b]; no collectives. Host pre-transposes X[b] -> [E, S], converts matmul
operands to bf16, and packs weights into wide slabs so the device issues only
7 input DMAs (scalar + sync HW queues in compute order).

Per-core dataflow (bf16 matmuls, fp32 PSUM accumulate / softmax math):
  XT slab [128, qc*2048 + c*512 + s] resident in SBUF
  QT/KT per head-pair per q-half: [128 dd, 512 q] (W chunk stationary);
      all QTs emitted before all KTs so the PE never queues behind the WK
      DMA; Q/K bias applied on ScalarE (Identity + per-partition bias AP)
  V = (X @ Wv + bv) -> 8 tiles [128 s, 8*(64+1)] with a ones column per head
      so the AV matmul also emits softmax denominators
  attention per q-chunk as ONE flat block stream across all 4 head-pairs,
  software-pipelined one k-block ahead (the pipeline crosses pair
  boundaries, so the PE never drains at a pair switch):
    score^T blocks [128 k, 2x512 q] on PE (k-blocks above the diagonal
    skipped, partial blocks column-trimmed), causal triangle fixed up on the
    PE itself (-1e9*I @ tri01), exp on ScalarE (scale=1/8 folded),
    AV accum O^T[65, q] with denom row 64.
  finalize per pair: copy denom row PSUM->SBUF (ScalarE for the last pair
    of each phase, DVE otherwise), reciprocal_approx_fast on DVE, GpSimd
    partition-broadcast, DVE scale; head-pair upper half placed via
    SBUF->SBUF DMA -- except the very last pair, whose projection reads the
    scaled halves directly via two K=64/65 matmuls (the K=65 ones row adds
    bo for free), so no DMA sits on the tail-critical path.
  Y[s-chunk] = sum_p O_pair^T-block^T @ Wo + bo: si 0-3 emitted during the
  interphase (overlapping qc=1 attention), si 4-7 at the tail with pair-0..2
  contributions emitted ahead of the finalize-gated splits; yp4/5 reuse the
  score-pipeline PSUM slots. bf16 y stores stream out on both the sync and
  scalar HW queues; the host converts back to fp32.
"""

import numpy as np
import ml_dtypes

import concourse.bass as bass
import concourse.tile as tile
import concourse.mybir as mybir
from concourse import bacc
from concourse import bass_utils

B, S, E = 8, 1024, 512
H, DK, DV = 8, 64, 64
HD = H * DK  # 512
P = 128
EC = E // P  # 4 contraction chunks over E
NPAIR = H // 2
NCORES = 8
F32 = mybir.dt.float32
BF16 = mybir.dt.bfloat16
NEG = -1.0e9

_COMPILED = None


def _body(nc, tc, const, work, ps, pb, d):
    # ---- const tiles + packed input DMAs ----
    XT = const.tile([P, 4096], BF16, tag="xt", name="XT")
    WQ = const.tile([P, 2048], BF16, tag="wq", name="WQ")
    WK = const.tile([P, 2048], BF16, tag="wk", name="WK")
    WV = const.tile([P, 2048], BF16, tag="wv", name="WV")
    WO = const.tile([P, 2048], BF16, tag="wo", name="WO")
    CB = const.tile([P, 2176], BF16, tag="cb", name="CB")
    FB = const.tile([P, 8], F32, tag="fb", name="FB")

    # everything on the sync HW queue in compute-dependency order (half-size
    # leading chunks so the first QT matmuls start early); only the small
    # bias/mask slabs go on the scalar engine's parallel queue
    nc.sync.dma_start(WQ[:, 0:1024], d["wq"][:, 0:1024])
    nc.sync.dma_start(XT[:, 0:1024], d["xt"][:, 0:1024])
    nc.sync.dma_start(XT[:, 1024:2048], d["xt"][:, 1024:2048])
    nc.sync.dma_start(WK[:, 0:1024], d["wk"][:, 0:1024])
    nc.sync.dma_start(WV[:], d["wv"][:])
    nc.sync.dma_start(XT[:, 2048:4096], d["xt"][:, 2048:4096])
    nc.sync.dma_start(WO[:], d["wo"][:])
    nc.scalar.dma_start(FB[:], d["fb"][:])
    nc.scalar.dma_start(WQ[:, 1024:2048], d["wq"][:, 1024:2048])
    nc.scalar.dma_start(WK[:, 1024:2048], d["wk"][:, 1024:2048])
    nc.scalar.dma_start(CB[:], d["cb"][:])

    bq_t = FB[:, 0:4]
    bk_t = FB[:, 4:8]
    negi = CB[:, 0:128]
    tri2 = CB[:, 128:384]
    bvb = CB[:, 384:896]
    bob = CB[:, 896:1408]
    # Wo rows for the tail pair's upper head, re-homed at partition base 0,
    # with bo as row DV (pairs with the ones row in the tail tmp tile)
    wo3b = CB[0:DV + 1, 1408:1920]
    # keep-mask (k <= q), duplicated for both heads of a pair
    keep2 = CB[:, 1920:2176]

    qt, kt, ot_sb, ot_tmp = {}, {}, {}, {}
    v_sb = [None] * 8

    def emit_qk_one(p, qc, which):
        W, bias, store, nm = ((WQ, bq_t, qt, "q") if which == "q"
                              else (WK, bk_t, kt, "k"))
        pp = ps.tile([P, 512], F32, tag="ps", name=f"{nm}p{p}{qc}")
        for c in range(EC):
            nc.tensor.matmul(
                pp[:], W[:, c * 512 + p * P:c * 512 + (p + 1) * P],
                XT[:, qc * 2048 + c * 512:qc * 2048 + (c + 1) * 512],
                start=(c == 0), stop=(c == EC - 1))
        t = const.tile([P, 512], BF16, tag=f"{nm}t{p}{qc}", name=f"{nm}t{p}{qc}")
        nc.scalar.activation(
            t[:], pp[:], mybir.ActivationFunctionType.Identity,
            bias=bias[:, p:p + 1])
        store[p, qc] = t

    def emit_qkt(p, qc):
        emit_qk_one(p, qc, "q")
        emit_qk_one(p, qc, "k")

    def emit_v(si):
        qc, sl = divmod(si, 4)
        vp = ps.tile([P, HD], F32, tag="ps", name=f"vp{si}")
        for c in range(EC):
            base = qc * 2048 + c * 512
            nc.tensor.matmul(
                vp[:], XT[:, base + sl * P:base + (sl + 1) * P],
                WV[:, c * 512:(c + 1) * 512],
                start=(c == 0), stop=(c == EC - 1))
        t = const.tile([P, H * 65], BF16, tag=f"v{si}", name=f"v{si}")
        t3 = t.rearrange("p (h c) -> p h c", c=65)
        nc.gpsimd.memset(t[:], 1.0)  # contiguous; leaves the per-head ones column
        nc.vector.tensor_add(
            t3[:, :, 0:DV],
            vp.rearrange("p (h c) -> p h c", c=DV),
            bvb.rearrange("p (h c) -> p h c", c=DV))
        v_sb[si] = t

    def emit_proj(si):
        qc, sl = divmod(si, 4)
        yp = ps.tile([P, E], F32, tag="ps", name=f"yp{si}")
        for p in range(NPAIR):
            nc.tensor.matmul(
                yp[:], ot_sb[p, qc][:, sl * P:(sl + 1) * P],
                WO[:, p * 512:(p + 1) * 512],
                start=(p == 0), stop=(p == NPAIR - 1), skip_group_check=True)
        yo = work.tile([P, E], BF16, tag="yo", name=f"yo{si}", bufs=4)
        nc.vector.tensor_add(yo[:], yp[:], bob[:])
        nc.sync.dma_start(d["y"][si * P:(si + 1) * P, :], yo[:])

    def emit_proj_tail():
        # si 4..7: pairs 0-2 emitted first (they overlap the last pair's
        # finalize chain), then the last pair via two K=64 matmuls straight
        # from the scaled halves (no SBUF->SBUF DMA on the tail path).
        # yp4/5 live in the score-pipeline psum slots (free by now); yp6/7
        # take pair-2's old accumulator banks.
        yps = {}
        for si in range(4, 8):
            sl = si - 4
            pool = pb if si < 6 else ps
            tag = "st" if si < 6 else "ps"
            yp = pool.tile([P, E], F32, tag=tag, name=f"yp{si}")
            yps[si] = yp

        def part(si, p):  # one pair-(0..2) contribution, K=128
            sl = si - 4
            nc.tensor.matmul(
                yps[si][:], ot_sb[p, 1][:, sl * P:(sl + 1) * P],
                WO[:, p * 512:(p + 1) * 512],
                start=(p == 0), stop=False, skip_group_check=True)

        def split(si, hb):  # last-pair halves, K=64/65 from the scaled temps
            sl = si - 4
            p3 = NPAIR - 1
            if hb == 0:
                nc.tensor.matmul(
                    yps[si][:], ot_sb[p3, 1][0:DV, sl * P:(sl + 1) * P],
                    WO[0:DV, p3 * 512:(p3 + 1) * 512],
                    start=False, stop=False, tile_position=(0, 0),
                    skip_group_check=True)
            else:
                # K=65: row DV of tmp is ones, row DV of wo3b is bo
                nc.tensor.matmul(
                    yps[si][:], ot_tmp[p3, 1][:, sl * P:(sl + 1) * P],
                    wo3b[:],
                    start=False, stop=True, tile_position=(0, 0),
                    skip_group_check=True)

        # pair-0..2 contributions first (they run during the last finalize
        # chain), then the gated splits in si pairs so early si groups close
        # while the PE still has split work left
        for si in (4, 5, 6, 7):
            part(si, 0)
            part(si, 1)
            part(si, 2)
        for si_pair in ((4, 5), (6, 7)):
            for si in si_pair:
                split(si, 0)
            for si in si_pair:
                split(si, 1)
        for si in range(4, 8):
            yo = work.tile([P, E], BF16, tag="yo", name=f"yo{si}", bufs=4)
            if si % 2 == 0:
                nc.vector.tensor_copy(yo[:], yps[si][:])
                nc.sync.dma_start(d["y"][si * P:(si + 1) * P, :], yo[:])
            else:
                nc.scalar.activation(yo[:], yps[si][:],
                                     mybir.ActivationFunctionType.Copy)
                nc.scalar.dma_start(d["y"][si * P:(si + 1) * P, :], yo[:])

    otps = {}

    def finalize(p, qc):
        # ---- finalize: O^T *= 1/denom (denom in row DV of otp); per-head
        # chains interleaved so DVE/GpSimd/DMA stages pipeline ----
        otp = otps[p, qc]
        ot = const.tile([P, 512], BF16, tag=f"ot{p}{qc}", name=f"ot{p}{qc}")
        rec, rb = {}, {}
        # last pair of either phase: ACT is about to idle, and this chain
        # gates the next phase's psum slots
        act_copy = (p == NPAIR - 1)
        tail_pair = (qc == 1 and p == NPAIR - 1)
        for hb in (0, 1):
            rrow = work.tile([1, 512], F32, tag="rrow", name=f"rrow{p}{qc}{hb}",
                             bufs=2)
            if act_copy:
                # ACT is idle once the phase's last exp retired; freeing DVE
                # here shortens the phase-critical reciprocal chain
                nc.scalar.activation(rrow[:], otp[hb][DV:DV + 1, :],
                                     mybir.ActivationFunctionType.Copy)
            else:
                nc.vector.tensor_copy(rrow[:], otp[hb][DV:DV + 1, :])
            rec[hb] = work.tile([1, 512], F32, tag="rec", name=f"rec{p}{qc}{hb}",
                                bufs=2)
            nc.vector.reciprocal_approx_fast(rec[hb][:], rrow[:])
            rb[hb] = work.tile([DV, 512], F32, tag="rb", name=f"rb{p}{qc}{hb}",
                               bufs=2)
            nc.gpsimd.partition_broadcast(rb[hb][:], rec[hb][:])
        nc.vector.tensor_mul(ot[0:DV, :], otp[0][0:DV, :], rb[0][:])
        # DVE cannot shift partitions: scale into a temp at base 0, then
        # SBUF->SBUF DMA into partitions 64-127 of the pair tile (skipped
        # for the tail pair, whose projection reads the temp directly and
        # picks up +bo through the ones row at partition DV)
        tmp = work.tile([DV + 1, 512], BF16, tag="ottmp",
                        name=f"ottmp{p}{qc}", bufs=2)
        nc.vector.tensor_mul(tmp[0:DV, :], otp[1][0:DV, :], rb[1][:])
        if tail_pair:
            nc.gpsimd.memset(tmp[DV:DV + 1, :], 1.0)
        else:
            nc.sync.dma_start(ot[DV:P, :], tmp[0:DV, :])
        ot_sb[p, qc] = ot
        ot_tmp[p, qc] = tmp

    def attn_phase(qc):
        # flat block stream across all pairs: the one-ahead score pipeline
        # crosses pair boundaries, so the PE never drains at a pair switch
        n_ki = 4 * (qc + 1)
        blocks = [(p, ki) for p in range(NPAIR) for ki in range(n_ki)]
        stps, stes = {}, {}

        def emit_score(p, ki):
            kc, kl = divmod(ki, 4)
            diag = (ki * P - qc * 512) >= 0
            off = max(ki * P - qc * 512, 0)
            # qc=1 diag blocks are masked post-exp on DVE instead (saves the
            # PE fixup matmuls); qc=0 keeps the PE fixup since every qc=0
            # block is diagonal and the DVE would gate the exp pipeline
            pe_fix = diag and qc == 0
            stp = pb.tile([P, 1024], F32, tag="st", name=f"st{p}{qc}{ki}")
            for hb in (0, 1):
                hp = slice(hb * DK, (hb + 1) * DK)
                nc.tensor.matmul(
                    stp[:, hb * 512 + off:(hb + 1) * 512],
                    kt[p, kc][hp, kl * P:(kl + 1) * P],
                    qt[p, qc][hp, off:],
                    start=True, stop=not pe_fix, tile_position=(hb * DK, 0),
                    skip_group_check=True)
            if pe_fix:
                # causal triangle applied on the PE itself (-1e9*I @ tri01
                # accumulated into the diagonal 128 columns)
                for hb in (0, 1):
                    nc.tensor.matmul(
                        stp[:, hb * 512 + off:hb * 512 + off + P],
                        negi[:], tri2[:, 0:P],
                        start=False, stop=True, skip_group_check=True)
            stps[p, ki] = (stp, off, diag and qc == 1)

        def emit_exp(p, ki):
            stp, off, dve_mask = stps[p, ki]
            ste = work.tile([P, 1024], BF16, tag="ste", name=f"ste{p}{qc}{ki}",
                            bufs=3)
            if off == 0:
                nc.scalar.activation(
                    ste[:], stp[:], mybir.ActivationFunctionType.Exp,
                    scale=0.125)
            else:
                stp3 = stp.rearrange("p (h q) -> p h q", h=2)[:, :, off:]
                ste3 = ste.rearrange("p (h q) -> p h q", h=2)[:, :, off:]
                nc.scalar.activation(
                    ste3, stp3, mybir.ActivationFunctionType.Exp, scale=0.125)
            if dve_mask:
                # zero exp'd scores above the diagonal (both heads at once);
                # DVE, not GpSimd: the GpSimd ALU is ~3x slower here and
                # gates every diagonal AV (measured 146us vs 105us)
                sv = ste.rearrange("p (h q) -> p h q", h=2)[:, :, off:off + P]
                nc.vector.tensor_mul(
                    sv, sv, keep2.rearrange("p (h q) -> p h q", h=2))
            stes[p, ki] = (ste, off)

        def emit_av(p, ki):
            ste, off = stes[p, ki]
            st_f, sp_f = (ki == 0), (ki == n_ki - 1)
            for hb in (0, 1):
                h = 2 * p + hb
                nc.tensor.matmul(
                    otps[p, qc][hb][:, off:], v_sb[ki][:, h * 65:h * 65 + 65],
                    ste[:, hb * 512 + off:(hb + 1) * 512],
                    start=st_f, stop=sp_f, skip_group_check=True)

        for idx, (p, ki) in enumerate(blocks):
            if ki == 0:
                otps[p, qc] = [ps.tile([DV + 1, 512], F32, tag="ps",
                                       name=f"otp{p}{qc}{hb}") for hb in (0, 1)]
            emit_score(p, ki)
            emit_exp(p, ki)
            if idx >= 1:
                pp, pk = blocks[idx - 1]
                emit_av(pp, pk)
                if pk == n_ki - 1:
                    finalize(pp, qc)
        emit_av(*blocks[-1])
        finalize(NPAIR - 1, qc)

    # ---- emission schedule (ps-tag rotation keeps otp pairs ping-ponging
    # between bank pairs; see allocation count comments) ----
    for p in range(NPAIR):            # ps allocs 0-7; all QTs first so the
        emit_qk_one(p, 0, "q")        # PE never queues behind the WK DMA
    for p in range(NPAIR):
        emit_qk_one(p, 0, "k")
    for si in range(4):               # 8-11
        emit_v(si)
    attn_phase(0)                     # 12-19 -> slots (0,1)/(2,3)/(0,1)/(2,3)
    emit_qkt(0, 1)                    # 20-21
    emit_qkt(1, 1)                    # 22-23
    emit_v(4)                         # 24
    emit_v(5)                         # 25
    emit_qkt(2, 1)                    # 26-27
    emit_qkt(3, 1)                    # 28-29
    emit_v(6)                         # 30
    emit_v(7)                         # 31
    for si in range(4):               # 32-35 (needs all qc=0 ot, finalized)
        emit_proj(si)
    attn_phase(1)                     # 36-43 -> slots (0,1)/(2,3)/(0,1)/(2,3)
    emit_proj_tail()                  # yp6/7 at ps 44,45 -> slots 0,1


def _build():
    nc = bacc.Bacc("TRN2", target_bir_lowering=False, debug=False)
    d = {
        "xt": nc.dram_tensor("xt", [P, 4096], BF16, kind="ExternalInput").ap(),
        "wq": nc.dram_tensor("wq", [P, 2048], BF16, kind="ExternalInput").ap(),
        "wk": nc.dram_tensor("wk", [P, 2048], BF16, kind="ExternalInput").ap(),
        "wv": nc.dram_tensor("wv", [P, 2048], BF16, kind="ExternalInput").ap(),
        "wo": nc.dram_tensor("wo", [P, 2048], BF16, kind="ExternalInput").ap(),
        "cb": nc.dram_tensor("cb", [P, 2176], BF16, kind="ExternalInput").ap(),
        "fb": nc.dram_tensor("fb", [P, 8], F32, kind="ExternalInput").ap(),
        "y": nc.dram_tensor("y", [S, E], BF16, kind="ExternalOutput").ap(),
    }
    with tile.TileContext(nc) as tc:
        with tc.tile_pool(name="const", bufs=1) as const, \
             tc.tile_pool(name="work", bufs=3) as work, \
             tc.tile_pool(name="ps", bufs=4, space="PSUM") as ps, \
             tc.tile_pool(name="pb", bufs=2, space="PSUM") as pb:
            _body(nc, tc, const, work, ps, pb, d)
    nc.compile()
    return nc


def get_nc():
    global _COMPILED
    if _COMPILED is None:
        _COMPILED = _build()
    return _COMPILED


def _prep_in_maps(X, Wq, bq, Wk, bk, Wv, bv, Wo, bo):
    f = np.float32
    bf = ml_dtypes.bfloat16

    def wslab(W):  # [H,E,Dk] -> [128, c*512 + (h*64+d)]
        Wr = np.transpose(np.asarray(W, f), (1, 0, 2)).reshape(E, HD)
        return np.ascontiguousarray(
            Wr.reshape(EC, P, HD).transpose(1, 0, 2).reshape(P, EC * HD).astype(bf))

    shared = {
        "wq": wslab(Wq),
        "wk": wslab(Wk),
        "wv": wslab(Wv),
        "wo": np.ascontiguousarray(
            np.asarray(Wo, f).reshape(EC, P, E).transpose(1, 0, 2)
            .reshape(P, EC * E).astype(bf)),
    }
    bq_t = np.asarray(bq, f).reshape(HD).reshape(NPAIR, P).T
    bk_t = np.asarray(bk, f).reshape(HD).reshape(NPAIR, P).T
    bvb = np.broadcast_to(np.asarray(bv, f).reshape(1, HD), (P, HD)).astype(bf)
    bob = np.broadcast_to(np.asarray(bo, f).reshape(1, E), (P, E)).astype(bf)
    kk = np.arange(P)[:, None]
    jj = np.arange(P)[None, :]
    shared["fb"] = np.ascontiguousarray(
        np.concatenate([bq_t, bk_t], axis=1).astype(f))
    negi = (np.eye(P, dtype=f) * NEG).astype(bf)
    tri01 = (kk > jj).astype(bf)
    # Wo rows 448:512 (tail pair, upper head) re-homed to partitions 0:64,
    # bo at partition DV (picked up by the ones row in the tail tmp tile)
    wo3b = np.zeros((P, E), dtype=bf)
    wo3b[0:DV] = np.asarray(Wo, f)[HD - DV:HD, :].astype(bf)
    wo3b[DV] = np.asarray(bo, f).reshape(E).astype(bf)
    keep01 = (kk <= jj).astype(bf)
    shared["cb"] = np.ascontiguousarray(
        np.concatenate([negi, tri01, tri01, bvb, bob, wo3b, keep01, keep01],
                       axis=1))

    Xf = np.asarray(X, f)
    in_maps = []
    for b in range(B):
        m = dict(shared)
        # xt slab: [128, qc*2048 + c*512 + s']
        m["xt"] = np.ascontiguousarray(
            Xf[b].T.reshape(EC, P, 2, 512).transpose(1, 2, 0, 3)
            .reshape(P, 4096).astype(bf))
        in_maps.append(m)
    return in_maps


def kernel(X, Wq, bq, Wk, bk, Wv, bv, Wo, bo):
    nc = get_nc()
    in_maps = _prep_in_maps(X, Wq, bq, Wk, bk, Wv, bv, Wo, bo)
    res = bass_utils.run_bass_kernel_spmd(nc, in_maps, core_ids=list(range(NCORES)))
    return np.stack([res.results[b]["y"] for b in range(B)], axis=0).astype(np.float32)


def run_traced(X, Wq, bq, Wk, bk, Wv, bv, Wo, bo):
    """Like kernel() but with NTFF profiling; returns (out, exec_time_ns)."""
    nc = get_nc()
    in_maps = _prep_in_maps(X, Wq, bq, Wk, bk, Wv, bv, Wo, bo)
    res = bass_utils.run_bass_kernel_spmd(
        nc, in_maps, core_ids=list(range(NCORES)), trace=True)
    out = np.stack([res.results[b]["y"] for b in range(B)], axis=0).astype(np.float32)
    return out, res.exec_time_ns
